# revision 1
# baseline (speedup 1.0000x reference)
"""LocalWindowAttention (block-causal) Trainium2 kernel, 8 NeuronCores.

Sharding: tensor-parallel over heads. Core c owns head-columns
[c*128, (c+1)*128) of the D=1024 hidden dim (2 heads x head_dim 64):
  - computes Q/K/V projections for its head slice (transposed layout),
  - block-causal attention for its 2 heads,
  - partial output projection with its 128 rows of Wo,
  - chunked ReduceScatter(add) sums partials; core c keeps rows
    [c*128,(c+1)*128) of final^T [1024, 2048]. Host reassembles.

All big matmuls run in float32r (fp32 with ~13-bit mantissa rounding on
the PE read path): 1 cycle/row for free dim >= 256 -- 4x faster than
plain fp32, ~32x more precise than bf16.

Attention runs in S^T layout (keys on partitions, queries free):
S^T tile = K_chunk @ Q^T. No max-subtraction needed (scores bounded),
and the softmax denominator comes free from a ones-column appended to
the V operand of the attn@V matmul (output row 64 = sum_k exp(s)).
The two heads are interleaved so the exp (ACT engine) of one head
hides behind the other head's matmuls, keeping the PE dense and the
HAM clock un-throttled. Query chunks are processed in descending
visibility order so each chunk's partial output projection and its
ReduceScatter slice overlap the remaining attention compute.
"""

import numpy as np

import concourse.bacc as bacc
import concourse.tile as tile
from concourse import mybir
from concourse.bass_utils import run_bass_kernel_spmd
from concourse.masks import make_identity

B, T, D = 1, 2048, 1024
H, HD, W = 16, 64, 128
N_CORES = 8
HS = D // N_CORES        # 128 head-columns per core (2 heads)
HPC = H // N_CORES       # heads per core
QW = 512                 # query-chunk width (free dim of S^T tiles)
NQ = T // QW             # 4 query chunks
NK = T // W              # 16 key chunks of 128
ND = D // 128            # 8 contraction chunks over D
SCALE = HD ** -0.5

F32 = mybir.dt.float32
F32R = mybir.dt.float32r
BF16 = mybir.dt.bfloat16
Exp = mybir.ActivationFunctionType.Exp

_compiled = {}


def _build():
    nc = bacc.Bacc("TRN2", target_bir_lowering=False, debug=False,
                   num_devices=N_CORES)
    xT_ap = nc.dram_tensor("xT", [D, T], F32R, kind="ExternalInput").ap()
    wq_ap = nc.dram_tensor("wq", [D, HS], F32R, kind="ExternalInput").ap()
    wk_ap = nc.dram_tensor("wk", [D, HS], F32R, kind="ExternalInput").ap()
    wv_ap = nc.dram_tensor("wv", [D, HS], F32R, kind="ExternalInput").ap()
    wo_ap = nc.dram_tensor("wo", [D, HS], F32R, kind="ExternalInput").ap()
    y_ap = nc.dram_tensor("y", [HS, T], F32, kind="ExternalOutput").ap()

    with tile.TileContext(nc) as tc:
        _body(tc, xT_ap, wq_ap, wk_ap, wv_ap, wo_ap, y_ap)
    nc.compile()
    return nc


def _body(tc, xT_ap, wq_ap, wk_ap, wv_ap, wo_ap, y_ap):
    nc = tc.nc
    from contextlib import ExitStack
    with ExitStack() as ctx:
        singles = ctx.enter_context(tc.tile_pool(name="singles", bufs=1))
        work = ctx.enter_context(tc.tile_pool(name="work", bufs=4))
        es_pool = ctx.enter_context(tc.tile_pool(name="es_pool", bufs=8))
        dram = ctx.enter_context(tc.tile_pool(name="dram", bufs=1, space="DRAM"))

        # ---- load inputs (weights first: small, unblock first matmuls) ----
        wq = singles.tile([128, ND, HS], F32R, tag="wq")
        wk = singles.tile([128, ND, HS], F32R, tag="wk")
        wv = singles.tile([128, ND, HS], F32R, tag="wv")
        nc.sync.dma_start(out=wq[:], in_=wq_ap.rearrange("(c p) m -> p c m", p=128))
        nc.sync.dma_start(out=wk[:], in_=wk_ap.rearrange("(c p) m -> p c m", p=128))
        nc.sync.dma_start(out=wv[:], in_=wv_ap.rearrange("(c p) m -> p c m", p=128))
        # x chunks 0-3 stream on the sync queue right behind the weights;
        # chunks 4-7 + wo go on the scalar engine's queue in parallel (ACT
        # is idle during the load phase)
        xts = []
        for d in range(ND):
            xt = singles.tile([128, T], F32R, tag=f"x{d}", name=f"xt{d}")
            eng = nc.sync if d < 4 else nc.scalar
            eng.dma_start(out=xt[:], in_=xT_ap[d * 128:(d + 1) * 128, :])
            xts.append(xt)
        wo = singles.tile([128, ND, HS], F32R, tag="wo")
        nc.scalar.dma_start(out=wo[:], in_=wo_ap.rearrange("(c p) m -> p c m", p=128))

        ident_f32 = singles.tile([128, 128], F32, tag="ident_f32")
        make_identity(nc, ident_f32)
        ident = singles.tile([128, 128], F32R, tag="ident")
        nc.vector.tensor_copy(ident[:], ident_f32[:])

        qT = singles.tile([128, T], F32R, tag="qT")
        kT = singles.tile([128, T], F32R, tag="kT")
        vT = singles.tile([128, T], F32R, tag="vT")
        # V in natural layout per head: [key 128, NK chunks, HD + ones col]
        vn = [singles.tile([128, NK, HD + 1], F32R, tag=f"vn{h}", name=f"vn{h}")
              for h in range(HPC)]
        outT = singles.tile([128, T], F32R, tag="outT")

        # ---- Q/K/V projections (transposed layout) ------------------------
        with tc.tile_pool(name="pp", bufs=2, space="PSUM") as pp:
            for t in range(NQ):
                ps_q = pp.tile([128, QW], F32, tag="q")
                ps_k = pp.tile([128, QW], F32, tag="k")
                ps_v = pp.tile([128, QW], F32, tag="v")
                cols = slice(t * QW, (t + 1) * QW)
                for d in range(ND):
                    f = (d == 0)
                    l = (d == ND - 1)
                    nc.tensor.matmul(ps_q[:], wq[:, d, :], xts[d][:, cols], start=f, stop=l)
                    nc.tensor.matmul(ps_k[:], wk[:, d, :], xts[d][:, cols], start=f, stop=l)
                    nc.tensor.matmul(ps_v[:], wv[:, d, :], xts[d][:, cols], start=f, stop=l)
                nc.vector.tensor_copy(qT[:, cols], ps_q[:])
                nc.vector.tensor_copy(kT[:, cols], ps_k[:])
                nc.vector.tensor_copy(vT[:, cols], ps_v[:])

        # ---- transpose V to natural layout, append ones column ------------
        ones = singles.tile([128, 1], F32, tag="ones")
        nc.vector.memset(ones[:], 1.0)
        for h in range(HPC):
            nc.vector.tensor_copy(vn[h][:, :, HD:],
                                  ones[:].unsqueeze(1).to_broadcast([128, NK, 1]))
        with tc.tile_pool(name="pt", bufs=3, space="PSUM") as pt:
            for tk in range(NK):
                ps_t = pt.tile([128, 128], F32R, tag="t")
                nc.tensor.transpose(
                    ps_t[:], vT[:, tk * W:(tk + 1) * W], ident[:])
                for h in range(HPC):
                    nc.vector.tensor_copy(vn[h][:, tk, :HD],
                                          ps_t[:, h * HD:(h + 1) * HD])

        # ---- attention + output projection, query chunks descending -------
        ag_in = [dram.tile([HS, 2 * QW], F32R, name=f"ag_in{j}")
                 for j in range(NQ // 2)]
        ag_out = [dram.tile([N_CORES, HS, 2 * QW], F32R, addr_space="Shared",
                            name=f"ag_out{j}") for j in range(NQ // 2)]

        with tc.tile_pool(name="pa", bufs=2, space="PSUM") as pa, \
             tc.tile_pool(name="po", bufs=2, space="PSUM") as po:
            for t in range(NQ):
                cols = slice(t * QW, (t + 1) * QW)
                n_tk = 4 * t + 4
                ps_o = [po.tile([HD + 1, QW], F32, tag=f"o{h}", name=f"ps_o{h}")
                        for h in range(HPC)]

                def s_exp(tk):
                    qs = max(0, (tk - 4 * t) * W)  # masked cols before qs
                    # both heads' scores go into one double-wide PSUM tile so
                    # a single ACT instruction computes both exps (lower ACT
                    # overhead keeps the scalar engine ahead of the PE)
                    ps_s = pa.tile([128, 2 * QW], F32, tag="s", name="ps_s")
                    for h in range(HPC):
                        hrows = slice(h * HD, (h + 1) * HD)
                        nc.tensor.matmul(
                            ps_s[:, h * QW + qs:(h + 1) * QW],
                            kT[hrows, tk * W:(tk + 1) * W],
                            qT[hrows, t * QW + qs:(t + 1) * QW],
                            start=True, stop=True)
                    e = es_pool.tile([128, 2 * QW], F32R, tag="es", name="es")
                    nc.scalar.activation(out=e[:, qs:], in_=ps_s[:, qs:],
                                         func=Exp, scale=SCALE)
                    return e

                def av(tk, e):
                    qs = max(0, (tk - 4 * t) * W)
                    for h in range(HPC):
                        nc.tensor.matmul(ps_o[h][:, qs:], vn[h][:, tk, :],
                                         e[:, h * QW + qs:(h + 1) * QW],
                                         start=(tk == 0), stop=(tk == n_tk - 1))

                # software pipeline: scores/exp run two key-chunks ahead of
                # the attn@V accumulation so the PE always has independent
                # matmuls queued while the ACT works (n_tk >= 4 always)
                pend = [s_exp(0), s_exp(1)]
                for tk in range(2, n_tk):
                    pend.append(s_exp(tk))
                    av(tk - 2, pend.pop(0))
                av(n_tk - 2, pend.pop(0))
                av(n_tk - 1, pend.pop(0))

                # normalize: rows 0..63 / row 64
                for h in range(HPC):
                    hrows = slice(h * HD, (h + 1) * HD)
                    r_sb = work.tile([1, QW], F32, tag="rsb")
                    nc.vector.tensor_copy(r_sb[:], ps_o[h][HD:, :])
                    rec1 = work.tile([1, QW], F32, tag="rec1")
                    nc.vector.reciprocal(out=rec1[:], in_=r_sb[:])
                    bc = work.tile([HD, QW], F32, tag="bc")
                    nc.gpsimd.partition_broadcast(bc[:], rec1[:])
                    nc.vector.tensor_mul(outT[hrows, cols], ps_o[h][:HD, :], bc[:])

                # kick off a pair AllGather after each odd chunk; consumers
                # are emitted after ALL attention so the PE never stalls on it
                if t % 2 == 1:
                    j = t // 2
                    pcols = slice((t - 1) * QW, (t + 1) * QW)
                    nc.sync.dma_start(out=ag_in[j][:], in_=outT[:, pcols])
                    nc.gpsimd.collective_compute(
                        "AllGather", mybir.AluOpType.bypass,
                        replica_groups=[list(range(N_CORES))],
                        ins=[ag_in[j].opt()], outs=[ag_out[j].opt()])

        # apply the full Wo to the gathered activations: for our 128 output
        # columns, final^T[c-slice, cols] = sum_c Wo[c-block, slice]^T @ outT_c
        with tc.tile_pool(name="pf", bufs=2, space="PSUM") as pf, \
             tc.tile_pool(name="gt_pool", bufs=3) as gt_pool:
            for j in range(NQ // 2):
                ps_y = [pf.tile([128, QW], F32, tag=f"y{i}", name=f"ps_y{i}")
                        for i in range(2)]
                for c in range(N_CORES):
                    g = gt_pool.tile([128, 2 * QW], F32R, tag="g", name="g")
                    nc.sync.dma_start(out=g[:], in_=ag_out[j][c])
                    for i in range(2):
                        nc.tensor.matmul(ps_y[i][:], wo[:, c, :],
                                         g[:, i * QW:(i + 1) * QW],
                                         start=(c == 0), stop=(c == N_CORES - 1))
                for i in range(2):
                    t = 2 * j + i
                    cols = slice(t * QW, (t + 1) * QW)
                    cy = work.tile([128, QW], F32, tag="cy")
                    nc.vector.tensor_copy(cy[:], ps_y[i][:])
                    nc.sync.dma_start(out=y_ap[:, cols], in_=cy[:])


def kernel(x, Wq, Wk, Wv, Wo):
    if "nc" not in _compiled:
        _compiled["nc"] = _build()
    nc = _compiled["nc"]

    xT = np.ascontiguousarray(x.reshape(T, D).T.astype(np.float32))
    in_maps = []
    for c in range(N_CORES):
        hs = slice(c * HS, (c + 1) * HS)
        in_maps.append({
            "xT": xT,
            "wq": np.ascontiguousarray(Wq[:, hs].astype(np.float32)),
            "wk": np.ascontiguousarray(Wk[:, hs].astype(np.float32)),
            "wv": np.ascontiguousarray(Wv[:, hs].astype(np.float32)),
            "wo": np.ascontiguousarray(Wo[:, hs].astype(np.float32)),
        })
    res = run_bass_kernel_spmd(nc, in_maps, list(range(N_CORES)))
    finalT = np.concatenate([res.results[c]["y"] for c in range(N_CORES)], axis=0)
    return np.ascontiguousarray(finalT.T).reshape(B, T, D)



# revision 6
# speedup vs baseline: 1.2379x; 1.2379x over previous
"""LocalWindowAttention (block-causal) Trainium2 kernel, 8 NeuronCores.

Sharding: tensor-parallel over heads. Core c owns head-columns
[c*128, (c+1)*128) of the D=1024 hidden dim (2 heads x head_dim 64):
  - computes Q/K/V projections for its head slice (transposed layout),
  - block-causal attention for its 2 heads,
  - per-query-chunk AllGather of the pre-Wo activations,
  - applies the full Wo to the gathered activations for its 128 output
    dims; core c keeps rows [c*128,(c+1)*128) of final^T [1024, 2048].
    Host reassembles.

v2 (vs the 215us baseline):
  - bf16 datapath everywhere except PSUM accumulation (fp32) and the
    final output (fp32): halves HBM traffic, enables FWL weight loads,
    and 2x DVE copy modes. Host-side casts are free (not HW-timed).
  - fully chunk-pipelined: x streams in 512-column chunks; chunk t's
    projections run while chunk t-1's attention runs, so the PE starts
    ~4us in and stays dense (no HAM re-throttle to 1.2 GHz).
  - per-chunk AllGather (4x 128KB bf16) kicked right after each chunk's
    normalize; the Wo-projection consumer for chunk t is emitted after
    attention of chunk t+1, hiding collective latency.
  - reciprocal_approx_fast for softmax denominators (DVE RECIPROCAL is
    ~8 cyc/elem; the approx op is ~5x faster and 18-bit accurate).
  - V transposed to natural layout per chunk with a shared-ones layout
    [1|hd0|1|hd1] so one strided DVE copy fills both heads and the
    denominator comes free as row 0 of the attn@V output.

Attention runs in S^T layout (keys on partitions, queries free):
S^T tile = K_chunk @ Q^T. No max-subtraction needed (scores bounded).
The two heads' score matmuls land in different PSUM banks with
contraction rows 0-63 / 64-127, so the PE runs them concurrently
(row-group tiling); one ACT instruction computes both heads' exp.
"""

import numpy as np
from ml_dtypes import bfloat16

import concourse.bacc as bacc
import concourse.tile as tile
from concourse import mybir
from concourse.bass_utils import run_bass_kernel_spmd
from concourse.masks import make_identity

B, T, D = 1, 2048, 1024
H, HD, W = 16, 64, 128
N_CORES = 8
HS = D // N_CORES        # 128 head-columns per core (2 heads)
HPC = H // N_CORES       # heads per core
QW = 512                 # query-chunk width (free dim of S^T tiles)
NQ = T // QW             # 4 query chunks
NK = T // W              # 16 key chunks of 128
ND = D // 128            # 8 contraction chunks over D
SCALE = HD ** -0.5

F32 = mybir.dt.float32
BF16 = mybir.dt.bfloat16
Exp = mybir.ActivationFunctionType.Exp

_compiled = {}


def _build():
    nc = bacc.Bacc("TRN2", target_bir_lowering=False, debug=False,
                   num_devices=N_CORES)
    xT_ap = nc.dram_tensor("xT", [D, T], BF16, kind="ExternalInput").ap()
    wq_ap = nc.dram_tensor("wq", [D, HS], BF16, kind="ExternalInput").ap()
    wk_ap = nc.dram_tensor("wk", [D, HS], BF16, kind="ExternalInput").ap()
    wv_ap = nc.dram_tensor("wv", [D, HS], BF16, kind="ExternalInput").ap()
    wo_ap = nc.dram_tensor("wo", [D, HS], BF16, kind="ExternalInput").ap()
    y_ap = nc.dram_tensor("y", [HS, T], F32, kind="ExternalOutput").ap()

    with tile.TileContext(nc) as tc:
        _body(tc, xT_ap, wq_ap, wk_ap, wv_ap, wo_ap, y_ap)
    nc.compile()
    return nc


def _body(tc, xT_ap, wq_ap, wk_ap, wv_ap, wo_ap, y_ap):
    nc = tc.nc
    from contextlib import ExitStack
    with ExitStack() as ctx:
        singles = ctx.enter_context(tc.tile_pool(name="singles", bufs=1))
        work = ctx.enter_context(tc.tile_pool(name="work", bufs=3))
        es_pool = ctx.enter_context(tc.tile_pool(name="es_pool", bufs=6))
        vt_pool = ctx.enter_context(tc.tile_pool(name="vt_pool", bufs=2))
        g_pool = ctx.enter_context(tc.tile_pool(name="g_pool", bufs=2))
        dram = ctx.enter_context(tc.tile_pool(name="dram", bufs=1, space="DRAM"))
        # PSUM budget (8 banks): scores 2x2 + attn@V accum 2 + aux 2
        pa = ctx.enter_context(tc.tile_pool(name="pa", bufs=2, space="PSUM"))
        po = ctx.enter_context(tc.tile_pool(name="po", bufs=1, space="PSUM"))
        paux = ctx.enter_context(tc.tile_pool(name="paux", bufs=2, space="PSUM"))

        # ---- input DMAs, ordered so chunk-0 projections start ASAP ----
        wq = singles.tile([128, ND, HS], BF16, tag="wq")
        wk = singles.tile([128, ND, HS], BF16, tag="wk")
        wv = singles.tile([128, ND, HS], BF16, tag="wv")
        wo = singles.tile([128, ND, HS], BF16, tag="wo")
        xcs = [singles.tile([128, ND, QW], BF16, tag=f"x{t}", name=f"xc{t}")
               for t in range(NQ)]
        x_r = xT_ap.rearrange("(c p) (t m) -> p c t m", p=128, t=NQ)
        nc.sync.dma_start(out=wq[:], in_=wq_ap.rearrange("(c p) m -> p c m", p=128))
        nc.sync.dma_start(out=xcs[0][:], in_=x_r[:, :, 0, :])
        nc.sync.dma_start(out=wk[:], in_=wk_ap.rearrange("(c p) m -> p c m", p=128))
        nc.sync.dma_start(out=wv[:], in_=wv_ap.rearrange("(c p) m -> p c m", p=128))
        for t in range(1, NQ):
            nc.sync.dma_start(out=xcs[t][:], in_=x_r[:, :, t, :])
        nc.sync.dma_start(out=wo[:], in_=wo_ap.rearrange("(c p) m -> p c m", p=128))

        ident_f32 = singles.tile([128, 128], F32, tag="ident_f32")
        make_identity(nc, ident_f32)
        ident = singles.tile([128, 128], BF16, tag="ident")
        nc.vector.tensor_copy(ident[:], ident_f32[:])

        qT = singles.tile([128, T], BF16, tag="qT")
        kT = singles.tile([128, T], BF16, tag="kT")
        # V natural layout: [key 128, NK, hd0|1|hd1|1]; head h's stationary
        # operand is vn[:, tk, 65h:65h+65] = [hd, ones] so row 64 of the
        # attn@V output is the softmax denominator.
        vn = singles.tile([128, NK, 2 * (HD + 1)], BF16, tag="vn")
        nc.vector.memset(vn[:, :, HD], 1.0)
        nc.vector.memset(vn[:, :, 2 * HD + 1], 1.0)
        outT = singles.tile([128, T], BF16, tag="outT")

        def proj(t):
            cols = slice(t * QW, (t + 1) * QW)
            for dst, w in ((qT, wq), (kT, wk), (None, wv)):
                ps = paux.tile([128, QW], F32, tag="aux", name=f"pj{t}")
                for d in range(ND):
                    nc.tensor.matmul(ps[:], w[:, d, :], xcs[t][:, d, :],
                                     start=(d == 0), stop=(d == ND - 1))
                if dst is not None:
                    nc.vector.tensor_copy(dst[:, cols], ps[:])
                else:
                    vt = vt_pool.tile([128, QW], BF16, tag="vt", name=f"vt{t}")
                    nc.vector.tensor_copy(vt[:], ps[:])
                    for j in range(4):
                        tk = 4 * t + j
                        ps_t = paux.tile([128, 128], BF16, tag="aux",
                                         name=f"tr{tk}")
                        nc.tensor.transpose(ps_t[:], vt[:, j * W:(j + 1) * W],
                                            ident[:])
                        src = ps_t[:].rearrange("p (h m) -> p h m", h=2)
                        dst3 = vn[:, tk, :].rearrange("p (h m) -> p h m", h=2)
                        nc.vector.tensor_copy(dst3[:, :, 0:HD], src[:])

        ag_in = [dram.tile([HS, QW], BF16, name=f"ag_in{t}") for t in range(NQ)]
        ag_out = [dram.tile([N_CORES, HS, QW], BF16, addr_space="Shared",
                            name=f"ag_out{t}") for t in range(NQ)]

        def outproj(t):
            cols = slice(t * QW, (t + 1) * QW)
            g = g_pool.tile([128, N_CORES, QW], BF16, tag="g", name=f"g{t}")
            nc.scalar.dma_start(out=g[:], in_=ag_out[t].rearrange("c p m -> p c m"))
            ps_y = paux.tile([128, QW], F32, tag="aux", name=f"py{t}")
            for c in range(N_CORES):
                nc.tensor.matmul(ps_y[:], wo[:, c, :], g[:, c, :],
                                 start=(c == 0), stop=(c == N_CORES - 1))
            cy = work.tile([128, QW], F32, tag="cy", name=f"cy{t}")
            nc.vector.tensor_copy(cy[:], ps_y[:])
            nc.scalar.dma_start(out=y_ap[:, cols], in_=cy[:])

        def attention(t):
            cols = slice(t * QW, (t + 1) * QW)
            n_tk = 4 * t + 4
            ps_o = [po.tile([HD + 1, QW], F32, tag=f"o{h}", name=f"ps_o{h}")
                    for h in range(HPC)]

            def s_exp(tk):
                qs = max(0, (tk - 4 * t) * W)  # masked cols before qs
                # both heads' scores in one double-wide PSUM tile: the two
                # matmuls use contraction rows 0-63 / 64-127 -> different
                # PE row groups -> concurrent; one ACT instruction exps both
                ps_s = pa.tile([128, 2 * QW], F32, tag="s", name="ps_s")
                for h in range(HPC):
                    hrows = slice(h * HD, (h + 1) * HD)
                    nc.tensor.matmul(
                        ps_s[:, h * QW + qs:(h + 1) * QW],
                        kT[hrows, tk * W:(tk + 1) * W],
                        qT[hrows, t * QW + qs:(t + 1) * QW],
                        start=True, stop=True)
                e = es_pool.tile([128, 2 * QW], BF16, tag="es", name="es")
                nc.scalar.activation(out=e[:, qs:], in_=ps_s[:, qs:],
                                     func=Exp, scale=SCALE)
                return e

            def av(tk, e):
                qs = max(0, (tk - 4 * t) * W)
                for h in range(HPC):
                    nc.tensor.matmul(ps_o[h][:, qs:],
                                     vn[:, tk, h * (HD + 1):(h + 1) * (HD + 1)],
                                     e[:, h * QW + qs:(h + 1) * QW],
                                     start=(tk == 0), stop=(tk == n_tk - 1))

            # scores/exp run two key-chunks ahead of the attn@V accumulation
            pend = [s_exp(0), s_exp(1)]
            for tk in range(2, n_tk):
                pend.append(s_exp(tk))
                av(tk - 2, pend.pop(0))
            av(n_tk - 2, pend.pop(0))
            av(n_tk - 1, pend.pop(0))

            # normalize: rows 0..63 of each ps_o scaled by 1/row64
            for h in range(HPC):
                den = work.tile([1, QW], F32, tag=f"den{h}", name=f"den{t}_{h}")
                nc.vector.tensor_copy(den[:], ps_o[h][HD:HD + 1, :])
                rec = work.tile([1, QW], F32, tag=f"rec{h}", name=f"rec{t}_{h}")
                nc.vector.reciprocal_approx_fast(rec[:], den[:])
                bc = work.tile([HD, QW], F32, tag=f"bc{h}", name=f"bc{t}_{h}")
                nc.gpsimd.partition_broadcast(bc[:], rec[:], channels=HD)
                nc.vector.tensor_mul(outT[h * HD:(h + 1) * HD, cols],
                                     ps_o[h][0:HD, :], bc[:])

            nc.sync.dma_start(out=ag_in[t][:], in_=outT[:, cols])
            nc.gpsimd.collective_compute(
                "AllGather", mybir.AluOpType.bypass,
                replica_groups=[list(range(N_CORES))],
                ins=[ag_in[t].opt()], outs=[ag_out[t].opt()])

        proj(0)
        for t in range(NQ):
            attention(t)
            if t < NQ - 1:
                proj(t + 1)
            if t >= 1:
                outproj(t - 1)
        outproj(NQ - 1)


def _make_in_maps(x, Wq, Wk, Wv, Wo):
    xT = np.asarray(x, np.float32).reshape(T, D).T
    xT = np.ascontiguousarray(xT).astype(bfloat16)
    in_maps = []
    for c in range(N_CORES):
        hs = slice(c * HS, (c + 1) * HS)
        in_maps.append({
            "xT": xT,
            "wq": np.ascontiguousarray(np.asarray(Wq, np.float32)[:, hs]).astype(bfloat16),
            "wk": np.ascontiguousarray(np.asarray(Wk, np.float32)[:, hs]).astype(bfloat16),
            "wv": np.ascontiguousarray(np.asarray(Wv, np.float32)[:, hs]).astype(bfloat16),
            "wo": np.ascontiguousarray(np.asarray(Wo, np.float32)[:, hs]).astype(bfloat16),
        })
    return in_maps


def kernel(x, Wq, Wk, Wv, Wo):
    if "nc" not in _compiled:
        _compiled["nc"] = _build()
    nc = _compiled["nc"]
    in_maps = _make_in_maps(x, Wq, Wk, Wv, Wo)
    res = run_bass_kernel_spmd(nc, in_maps, list(range(N_CORES)))
    finalT = np.concatenate([res.results[c]["y"] for c in range(N_CORES)], axis=0)
    return np.ascontiguousarray(finalT.T).reshape(B, T, D).astype(np.float32)
